# revision 18
# baseline (speedup 1.0000x reference)
"""Causal single-head attention (b=4, n=2048, d=1024, fp32) on 8 TRN2 NeuronCores.

Sharding v3 — uniform padded zig-zag q-split. Core c = (batch c//2, role c%2).
Each role owns 8 of the 16 query subtiles of its batch (zig-zag interleaved,
see ROLE_SUBTILES); every core produces out rows for its own 1024 queries
with the FULL 1024 features.

The SPMD program is identical on all cores; the role only changes host-side
data: which columns land in xq (own queries), the causal masks, and where
host scatters the output rows. Causal work is padded to the elementwise-max
envelope across the two roles (AV_ENV) so both roles run the same instruction
stream; mask data zeroes the padding.

Per core pipeline (all matmuls bf16 -> fp32 PSUM):
  Scores use the algebraic fold S = x (W_q^T W_k) x^T: the host precomputes
  M = W_q^T W_k in fp32 during sharding prep, the kernel computes
  zq[b,q] = M^T xq^T (own 1024 q, replaces BOTH the Q and K projections) and
  contracts sT[k,q] against the resident x^T over b -- no K projection at all.
  P = exp(sT/32) * mask (no max subtraction; scaled scores are in [-2.6, 2.6]);
  row sums accumulate on the DVE across k-chunks, finished by one tiny
  ones-matmul per 128-query block. The value side is folded the same way:
  U^T[b,q] = x^T P (causal at q-half granularity) and out[q,o] = (U W_v^T)/l.

v3 layout: every input ships partition-major ([128, free]) so DMA runs are
2-16 KiB contiguous per partition (the early input phase is descriptor-rate
bound at 1 KiB runs). Masks carry only the 16 chunk slots that are not
all-ones for both roles. Output is written bf16 and widened on host.
"""

import os
import sys

if os.path.isdir("/opt/trn_rl_repo") and "/opt/trn_rl_repo" not in sys.path:
    sys.path.insert(0, "/opt/trn_rl_repo")

import numpy as np
import ml_dtypes

BF16 = ml_dtypes.bfloat16
F8E4 = ml_dtypes.float8_e4m3  # TRN fp8_e4: bias 7, max +-240, has inf

B, N, D = 4, 2048, 1024
NCORES = 8
P = 128
QT = 512
HQ = QT // 2       # 256: q-half granularity of the causal envelope
NKC = N // P       # 16 k chunks
NDC = D // P       # 8 d chunks
NPR = NDC // 2     # 4 d-chunk PAIRS (fp8 DoubleRow contraction granularity)
NQ_OWN = 1024      # own queries per core
NMK = 16           # mask slots shipped (chunks not all-ones for both roles)
SCALE = 1.0 / 32.0
# fp8 scales for the S = zq . x^T matmul (e4m3 max +-240; zq std ~0.33,
# x std 1 -> scaled operands stay well inside the normal range)
ZQ_SC = 16.0
X_SC = 32.0
S_DESC = 1.0 / (ZQ_SC * X_SC)

# Zig-zag assignment of the 16 query subtiles (128 rows each) to the two
# roles, chosen so the elementwise-max envelope across roles is minimal:
# slot0 = own subtiles drawn from {0..7}, slot1 from {8..15}.
ROLE_SUBTILES = {
    0: (0, 3, 4, 7, 8, 11, 12, 15),
    1: (1, 2, 5, 6, 9, 10, 13, 14),
}
# k-chunk envelope per (slot, subtile position): max over both roles of the
# causally-needed chunk count for the subtile each role places there.
AV_ENV = ((2, 4, 6, 8), (10, 12, 14, 16))
# First chunk per (slot, half) that is NOT all-ones for both roles: chunks
# below the threshold skip the mask multiply (and ship no mask data).
MASK_TH = ((0, 4), (8, 12))

_CACHE = {}


def _build_module():
    from concourse import bacc
    import concourse.tile as tile
    import concourse.mybir as mybir

    bf = mybir.dt.bfloat16
    f8 = mybir.dt.float8e4
    f32 = mybir.dt.float32
    Exp = mybir.ActivationFunctionType.Exp
    DR = mybir.MatmulPerfMode.DoubleRow

    nc = bacc.Bacc("TRN2", target_bir_lowering=False, debug=False, num_devices=NCORES)

    # All inputs partition-major: [128, ...] with multi-KiB contiguous runs.
    m_d = nc.dram_tensor("m", [P, NDC * NDC * P], bf, kind="ExternalInput")
    xq_d = nc.dram_tensor("xq", [P, 2 * NDC * QT], bf, kind="ExternalInput")
    # x^T for the scores contraction ships as e4m3 (x * 32), packed in
    # DoubleRow pair layout per k-chunk [p, kc, pr, j, k]: feature
    # d = (2*pr + j)*128 + p, key n = kc*128 + k.
    xT_d = nc.dram_tensor("xT", [P, NKC * NPR * 2 * P], f8, kind="ExternalInput")
    xN_d = nc.dram_tensor("xN", [P, NKC * D], bf, kind="ExternalInput")
    wv_d = nc.dram_tensor("wv", [P, NDC * D], bf, kind="ExternalInput")
    mk_d = nc.dram_tensor("masks", [P, NMK * QT], bf, kind="ExternalInput")
    out_d = nc.dram_tensor("out", [NQ_OWN, D], bf, kind="ExternalOutput")

    # m packed [p, g, dc, b2, j]: group g owns output-feature chunks
    # bt = 2g, 2g+1; per (g, dc) the 256 columns are contiguous so the
    # zq stream consumes the m DMA incrementally (dc-minor).
    m_r = m_d.ap().rearrange("p (g dc b2 j) -> p g dc b2 j", g=NPR, dc=NDC, b2=2)
    xq_r = xq_d.ap().rearrange("p (qt dc q) -> p qt dc q", qt=2, dc=NDC)
    xN_r = xN_d.ap().rearrange("p (kc b) -> p kc b", kc=NKC)
    wv_r = wv_d.ap().rearrange("p (dc o) -> p dc o", dc=NDC)
    mk_r = mk_d.ap().rearrange("p (j q) -> p j q", j=NMK)
    out_r = out_d.ap().rearrange("(s p) o -> p s o", p=P)

    with tile.TileContext(nc) as tc:
        with tc.tile_pool(name="pers", bufs=1) as pers:
            zq = pers.tile([P, NPR, 2, NQ_OWN], f8, tag="zq")
            xT = pers.tile([P, NKC, NPR, 2, P], f8, tag="xT")
            xN = pers.tile([P, NKC, D], bf, tag="xN")
            wv = pers.tile([P, NDC, D], bf, tag="wv")
            mks = pers.tile([P, NMK, QT], bf, tag="masks")
            ones = pers.tile([P, 1], bf, tag="ones")

            nc.vector.memset(ones[:], 1.0)

            # ---- zq projection ----
            with (
                tc.tile_pool(name="wp", bufs=1) as wp,
                tc.tile_pool(name="xsp", bufs=1) as xsp,
                tc.tile_pool(name="warm", bufs=1, space="PSUM") as warmps,
                tc.tile_pool(name="psA", bufs=4, space="PSUM") as psA,
            ):
                m = wp.tile([P, NPR, NDC, 2, P], bf, tag="m")
                xq = xsp.tile([P, 2, NDC, QT], bf, tag="xq")
                # PE pre-warm while the first DMAs land (HAM ramp). memset on
                # gpsimd: it exits the runtime preamble ~1.5us before DVE, so
                # the warm matmuls can fire the moment PE's queue opens.
                wsrc = pers.tile([P, QT], bf, tag="wsrc")
                nc.gpsimd.memset(wsrc[:], 0.0)
                wps = warmps.tile([P, QT], f32, tag="warm")
                for _ in range(3):
                    nc.tensor.matmul(wps, wsrc[:, :P], wsrc[:], start=True, stop=True)

                # DMA plan: the sync ring carries ONLY the zq-critical m/xq
                # stream (4MB) in consumption order at full bandwidth -- a
                # ring's transfers drain FIFO through one hw queue, and a
                # second active queue halves the critical stream's arrival
                # rate. The attention-phase bulk (xT/masks/xN/wv, 9MB) issues
                # on the scalar ring but only AFTER the first zq casts (see
                # the qt/g loop below), by which point the critical ramp is
                # done; everything bulk still lands ~15-20us before its
                # consumer because zq itself occupies the PE for ~28us.
                m_f = m[:].rearrange("p g dc b2 j -> p (g dc b2 j)")
                xq_f = xq[:].rearrange("p qt dc q -> p (qt dc q)")
                nc.sync.dma_start(xq_f[:, 0:2048], xq_d.ap()[:, 0:2048])
                nc.sync.dma_start(m_f[:, 0:2048], m_d.ap()[:, 0:2048])
                nc.sync.dma_start(xq_f[:, 2048:4096], xq_d.ap()[:, 2048:4096])
                nc.sync.dma_start(m_f[:, 2048:4096], m_d.ap()[:, 2048:4096])
                nc.sync.dma_start(m_f[:, 4096:6144], m_d.ap()[:, 4096:6144])
                nc.sync.dma_start(m_f[:, 6144:8192], m_d.ap()[:, 6144:8192])
                nc.sync.dma_start(xq_f[:, 4096:8192], xq_d.ap()[:, 4096:8192])
                xT_f = xT[:].rearrange("p kc pr j k -> p (kc pr j k)")
                XTC = NPR * 2 * P  # 1024 fp8 elems per k-chunk per partition
                mks_f = mks[:].rearrange("p j q -> p (j q)")
                xN_f = xN[:].rearrange("p kc b -> p (kc b)")
                wv_f = wv[:].rearrange("p dc o -> p (dc o)")

                # zq projection: zq[b, q] = M^T xq^T (own 1024 q), streamed as
                # 8 groups of 2 output-feature chunks (2 PSUM banks/group, so
                # the 4-buf pool keeps one group of lookahead) with the dc
                # contraction outermost inside a group: the first matmul only
                # needs 256 m columns + 512 xq columns on-chip.
                for qt in range(2):
                    for g in range(NPR):
                        pss = [
                            psA.tile([P, QT], f32, tag="proj", name=f"pj{qt}{g}{b}")
                            for b in range(2)
                        ]
                        for dc in range(NDC):
                            for b2 in range(2):
                                nc.tensor.matmul(
                                    pss[b2],
                                    m[:, g, dc, b2, :],
                                    xq[:, qt, dc, :],
                                    start=(dc == 0),
                                    stop=(dc == NDC - 1),
                                )
                        # scalar-engine cast to e4m3 (x ZQ_SC): the DVE is the
                        # attention phase's busy engine; zq casts queued there
                        # delay slot-1 scores behind mask-muls and uT casts.
                        for b2 in range(2):
                            nc.scalar.mul(
                                zq[:, g, b2, qt * QT : (qt + 1) * QT],
                                pss[b2], ZQ_SC,
                            )
                        if qt == 0 and g == 0:
                            # bulk input DMAs ride the ACT ring here, after
                            # the zq-critical ramp has cleared the sync queue
                            nc.scalar.dma_start(
                                xT_f[:, : 8 * XTC], xT_d.ap()[:, : 8 * XTC]
                            )
                            nc.scalar.dma_start(
                                mks_f[:, : 8 * QT], mk_d.ap()[:, : 8 * QT]
                            )
                            nc.scalar.dma_start(
                                xN_f[:, : NKC * D // 2],
                                xN_d.ap()[:, : NKC * D // 2],
                            )
                            nc.scalar.dma_start(wv_f, wv_d.ap())
                            nc.scalar.dma_start(
                                xT_f[:, 8 * XTC :], xT_d.ap()[:, 8 * XTC :]
                            )
                            nc.scalar.dma_start(
                                mks_f[:, 8 * QT :], mk_d.ap()[:, 8 * QT :]
                            )
                            nc.scalar.dma_start(
                                xN_f[:, NKC * D // 2 :],
                                xN_d.ap()[:, NKC * D // 2 :],
                            )

            # ---- attention ----
            with (
                tc.tile_pool(name="stps", bufs=2, space="PSUM") as stps,
                tc.tile_pool(name="smps", bufs=2, space="PSUM") as smps,
                tc.tile_pool(name="psU", bufs=2, space="PSUM") as psU,
                tc.tile_pool(name="outp", bufs=2, space="PSUM") as outp,
                tc.tile_pool(name="pTp", bufs=2) as pTp,
                tc.tile_pool(name="uTp", bufs=1) as uTp,
                tc.tile_pool(name="rap", bufs=2) as rap,
                tc.tile_pool(name="outst", bufs=2) as outst,
                tc.tile_pool(name="rcpp", bufs=8) as rcpp,
            ):
                uT = uTp.tile([P, NDC, NQ_OWN], bf, tag="uT")
                for slot in range(2):
                    sheet = pTp.tile([P, NKC, QT], bf, tag="sheet")
                    racc = rap.tile([P, QT], bf, tag="racc")
                    # scores at q-half (256) granularity: each half only needs
                    # chunks up to its own causal envelope (= AV_ENV[slot][2h+1])
                    for h in range(2):
                        nk = AV_ENV[slot][2 * h + 1]
                        th = MASK_TH[slot][h]
                        hq = slice(h * HQ, (h + 1) * HQ)
                        for c in range(nk):
                            ps = stps.tile([P, HQ], f32, tag="st")
                            for pr in range(NPR):
                                nc.tensor.matmul(
                                    ps,
                                    xT[:, c, pr, :, :],
                                    zq[:, pr, :, slot * QT + h * HQ :
                                       slot * QT + (h + 1) * HQ],
                                    start=(pr == 0),
                                    stop=(pr == NPR - 1),
                                    perf_mode=DR,
                                )
                            nc.scalar.activation(
                                sheet[:, c, hq], ps, Exp, bias=0.0,
                                scale=SCALE * S_DESC,
                            )
                            if c >= th:
                                nc.vector.tensor_mul(
                                    sheet[:, c, hq], sheet[:, c, hq], mks[:, c, hq]
                                )
                            # row-sum partials ride the DVE (k-lane partials;
                            # a single ones-matmul per 128-q block finishes).
                            if c == 1:
                                nc.vector.tensor_add(
                                    racc[:, hq], sheet[:, 0, hq], sheet[:, 1, hq]
                                )
                            elif c >= 2:
                                nc.vector.tensor_add(
                                    racc[:, hq], racc[:, hq], sheet[:, c, hq]
                                )
                    # Per half: U^T (fills the ACT exp/mask lag of the other
                    # half), then this half's row sums and output columns.
                    for h in range(2):
                        nk = AV_ENV[slot][2 * h + 1]
                        hq = slice(h * HQ, (h + 1) * HQ)
                        # row sums + reciprocals first: r is ready well before
                        # the out chains (the reciprocal queues on the DVE
                        # behind the uT casts otherwise, stalling the PE).
                        rs = {}
                        for j in (2 * h, 2 * h + 1):
                            sm = smps.tile([P, 1], f32, tag="sm")
                            nc.tensor.matmul(
                                sm, racc[:, j * P : (j + 1) * P], ones[:],
                                start=True, stop=True,
                            )
                            r = rcpp.tile([P, 1], f32, tag="rcp", name=f"r{slot}{j}")
                            nc.vector.reciprocal(r[:], sm)
                            rs[j] = r
                        # U^T[b, q] = x^T P (causal at q-half granularity)
                        for bt in range(NDC):
                            ps = psU.tile([P, HQ], f32, tag="ut")
                            for c in range(nk):
                                nc.tensor.matmul(
                                    ps,
                                    xN[:, c, bt * P : (bt + 1) * P],
                                    sheet[:, c, hq],
                                    start=(c == 0),
                                    stop=(c == nk - 1),
                                )
                            nc.vector.tensor_copy(
                                uT[:, bt, slot * QT + h * HQ :
                                   slot * QT + (h + 1) * HQ],
                                ps,
                            )
                        for j in (2 * h, 2 * h + 1):
                            r = rs[j]
                            # out[q, o] = U Wv^T, normalized by the row sums
                            s_idx = slot * 4 + j
                            qs = slice(slot * QT + j * P, slot * QT + (j + 1) * P)
                            ot = outst.tile([P, D], bf, tag="ot")
                            last = slot == 1 and j == 3
                            for oh in range(2):
                                ps = outp.tile([P, QT], f32, tag="out")
                                for bc in range(NDC):
                                    nc.tensor.matmul(
                                        ps,
                                        uT[:, bc, qs],
                                        wv[:, bc, oh * QT : (oh + 1) * QT],
                                        start=(bc == 0),
                                        stop=(bc == NDC - 1),
                                    )
                                if oh == 0:
                                    nc.scalar.mul(ot[:, :QT], ps, r[:])
                                    nc.sync.dma_start(
                                        out_r[:, s_idx, :QT], ot[:, :QT]
                                    )
                                elif not last:
                                    nc.vector.tensor_scalar_mul(ot[:, QT:], ps, r[:])
                                    nc.sync.dma_start(
                                        out_r[:, s_idx, QT:], ot[:, QT:]
                                    )
                                else:
                                    # final tile: quarter-granularity so the
                                    # last bytes ship as early as possible
                                    for qh in range(2):
                                        cs = slice(QT + qh * HQ, QT + (qh + 1) * HQ)
                                        if qh == 0:
                                            nc.vector.tensor_scalar_mul(
                                                ot[:, cs], ps[:, qh * HQ :
                                                              (qh + 1) * HQ], r[:]
                                            )
                                            nc.sync.dma_start(
                                                out_r[:, s_idx, cs], ot[:, cs]
                                            )
                                        else:
                                            nc.scalar.mul(
                                                ot[:, cs], ps[:, qh * HQ :
                                                              (qh + 1) * HQ], r[:]
                                            )
                                            # scalar-ring issue runs parallel
                                            # with sync's qh=0 issue
                                            nc.scalar.dma_start(
                                                out_r[:, s_idx, cs], ot[:, cs]
                                            )

    nc.compile()
    return nc


def _pack_pm(a):
    """[G*128, C] row-major -> [128, G*C] partition-major (2KB+ runs)."""
    g = a.shape[0] // P
    return np.ascontiguousarray(
        a.reshape(g, P, -1).transpose(1, 0, 2).reshape(P, -1)
    )


def _masks_np(role):
    subs = ROLE_SUBTILES[role]
    k = np.arange(P)[:, None]
    # original global query index for each local q column, per slot
    qg = []
    for slot in range(2):
        og = np.empty(QT, dtype=np.int64)
        for j in range(4):
            s = subs[slot * 4 + j]
            og[j * P : (j + 1) * P] = s * P + np.arange(P)
        qg.append(og[None, :])
    ms = []
    for c in range(NMK):
        ms.append(P * c + k <= qg[0 if c < 8 else 1])
    arr = np.stack(ms).astype(BF16)           # [16, 128, 512]
    return np.ascontiguousarray(arr.transpose(1, 0, 2).reshape(P, -1))


def get_module():
    if "nc" not in _CACHE:
        _CACHE["nc"] = _build_module()
    return _CACHE["nc"]


def make_in_maps(x, W_q, W_k, W_v):
    x = np.asarray(x, dtype=np.float32)
    W_q = np.asarray(W_q, dtype=np.float32)
    W_k = np.asarray(W_k, dtype=np.float32)
    # scores fold: S = x (W_q^T W_k) x^T -- M computed once in fp32
    m = (W_q.T @ W_k).astype(BF16)
    # m packed [p, g, dc, b2, j] = m[dc*128+p, (2g+b2)*128+j]
    m_p = np.ascontiguousarray(
        m.reshape(NDC, P, NPR, 2, P).transpose(1, 2, 0, 3, 4).reshape(P, -1)
    )
    wvT = np.asarray(W_v, dtype=np.float32).T.astype(BF16)
    wv_p = _pack_pm(wvT)
    masks = [_masks_np(r) for r in range(2)]
    in_maps = []
    for bidx in range(B):
        xb = x[bidx].astype(BF16)             # [N, D]
        # fp8 x^T in DoubleRow pair layout [p, kc, pr, j, k]:
        # value = x[kc*128+k, (2*pr+j)*128+p] * X_SC as e4m3
        x8 = (x[bidx] * X_SC).astype(F8E4)    # [N, D] from fp32
        xT_p = np.ascontiguousarray(
            x8.reshape(NKC, P, NPR, 2, P).transpose(4, 0, 2, 3, 1).reshape(P, -1)
        )
        xN_p = _pack_pm(xb)
        for r in range(2):
            qg = np.concatenate(
                [np.arange(s * P, (s + 1) * P) for s in ROLE_SUBTILES[r]]
            )
            xqsel = np.ascontiguousarray(xb[qg].T)    # [D, 1024]
            # xq packed [p, qt, dc, q] = xqsel[dc*128+p, qt*512+q]
            xq_p = np.ascontiguousarray(
                xqsel.reshape(NDC, P, 2, QT).transpose(1, 2, 0, 3).reshape(P, -1)
            )
            in_maps.append(
                {
                    "m": m_p,
                    "xq": xq_p,
                    "xT": xT_p,
                    "xN": xN_p,
                    "wv": wv_p,
                    "masks": masks[r],
                }
            )
    return in_maps


def kernel(x, W_q, W_k, W_v):
    from concourse.bass_utils import run_bass_kernel_spmd

    nc = get_module()
    in_maps = make_in_maps(x, W_q, W_k, W_v)
    res = run_bass_kernel_spmd(
        nc,
        in_maps,
        list(range(NCORES)),
        trace=bool(int(os.environ.get("KERNEL_TRACE", "0"))),
    )
    _CACHE["last_result"] = res
    out = np.empty((B, N, D), dtype=np.float32)
    for c in range(NCORES):
        b, r = c // 2, c % 2
        res_out = np.asarray(res.results[c]["out"]).astype(np.float32)
        for i, s in enumerate(ROLE_SUBTILES[r]):
            out[b, s * P : (s + 1) * P, :] = res_out[i * P : (i + 1) * P]
    return out

